# revision 27
# baseline (speedup 1.0000x reference)
"""Trainium2 Bass kernel: receptive-field one-hot time encoder.

reference semantics:
    t_spike[n, b] = clip(int(scaling[n] * |x[b] - center[n]|), 0, T-1)
    out[t, b, n]  = 1.0 if t == t_spike[n, b] else 0.0       # [T, B, N] f32

Data-parallel over 8 cores: x and the output batch axis are sharded into 8
contiguous blocks of B_c = 16384; center/scaling are replicated.

Per-core layout: x viewed as [128 partitions, 128 cols]; all per-(b, n)
quantities live in [128, 2048] tiles with free index f*16+n, which makes each
output t-slice a single fully contiguous 1 MiB DRAM store ([T, B_c, N] with
b = p*128+f).

The kernel is DMA-write-bound: 64 MiB of output per core at 360 GB/s is
~186.4 us, so everything else is organized to keep the DMA engines saturated
from the earliest possible instant:

  - a small input DMA (params + x), split so the first chunk's x lands first;
  - an exact floor chain (DVE + one ACT hop; all ops HW-valid -- the DVE
    tensor_scalar has no mod/abs_max/floor on real silicon):
        diff = x - center                  (tensor_tensor, broadcast APs)
        u    = |diff|                      (ACT Abs)
        v    = u * scaling                 (tensor_tensor)
        r    = (v + 2^23) - 2^23           (dual-op tensor_scalar: nearest int)
        tsp  = r - (r > v)                 (exact floor for 0 <= v < 2^23 under
                                            any nearest rounding of the add)
    one-hot rows: t=0 is_lt(v,1), t=63 is_ge(v,63), inner is_equal(tsp,t);
    (HEAD_DIRECT can instead compare x against host-bisected per-n
    thresholds for t=0/63 -- exact but 3 ops, off by default: the extra DVE
    serial time delays the ramp rows more than the head gains);
  - x-columns processed in a ramp of growing chunks; chunk c emits one-hot
    row pieces for t = 0, 63, 1..RAMPS[c] over its own columns, with the next
    chunk's prep ops interleaved between emissions (plus tile-scheduler wait
    hints) so the DMA backlog never drains during prep bursts;
  - rows RAMPS[c] < t <= RAMPS[c-1] get a single complement piece covering
    the later chunks' columns once the full tsp tile exists, then rows
    t > RAMPS[0] go out as single full-width DMAs;
  - early DMAs alternate between the SP/Act HWDGE paths and the Pool SWDGE
    path so descriptor generation never starves the DMA engines.

All compares are exact on the f32 lattice, so outputs match the jax reference
bit-for-bit (relative error 0.0, verified on HW).
"""

import sys

if "/opt/trn_rl_repo" not in sys.path:
    sys.path.insert(0, "/opt/trn_rl_repo")

import numpy as np

import concourse.tile as tile
from concourse import bacc, mybir
from concourse.alu_op_type import AluOpType
from concourse.bass_utils import run_bass_kernel_spmd

B, N, T = 131072, 16, 64
NCORES = 8
BC = B // NCORES  # 16384 per core
P = 128
F = BC // P  # 128 x-columns per partition
FN = F * N  # 2048 free elems per output row
HEAD_DIRECT = []                # chunks whose o0/o63 compare x directly


def _npar():
    return 6 * N if HEAD_DIRECT else 2 * N

# --- tunables -------------------------------------------------------------
CHUNK_COLS = [16, 24, 32, 56]   # x-column ramp (must sum to F)
RAMPS = [14, 10, 6, 2]          # per-chunk ramp rows t=1..RAMPS[c] (decreasing)
INTERLEAVE = 2                  # ramp emissions between interleaved prep ops
OUT_BUFS = 16
IN_SPLIT = 16                   # None = one input DMA; else split after this many x cols
SHARE_TAGS = True               # share prep-tile tags across chunks (WAR serialization)
PP_BUFS = 1                     # ring depth of the prep-tile pool
WAIT_HINTS = [None, 0.005, 0.009, 0.013]  # per-chunk scheduler wait hint (ms)
PREP_ORDER = ["diff", "u", "v", "r", "o0", "corr", "o63", "tsp"]
DIRECT_ORDER = ["o0", "o63", "diff", "u", "v", "r", "corr", "tsp"]
ABS_MODE = "act"                # "dve": max(diff,-diff); "act": ACT Abs
EARLY_ENGINES = ["pool", "sp", "act"]   # issue ring for chunk-piece DMAs
STEADY_ENGINES = ["sp", "act"]          # issue ring for complement/full rows

f32 = mybir.dt.float32

_cached = {}


def build_program(repeat=1):
    nc = bacc.Bacc(
        "TRN2", target_bir_lowering=False, debug=False, enable_asserts=False
    )
    # packed: [:, :N]=center, [:, N:2N]=scaling, [:, 2N:6N]=L0,H0,L63,H63,
    # [:, 6N:]=x
    NPAR = _npar()
    pk_d = nc.dram_tensor("packed", [P, NPAR + F], f32, kind="ExternalInput").ap()
    out_d = nc.dram_tensor("out", [T, P, FN], f32, kind="ExternalOutput").ap()

    nchunks = len(CHUNK_COLS)
    for order, direct in ((PREP_ORDER, False), (DIRECT_ORDER, True)):
        po = list(order)
        need_after = {
            "u": ["diff"] + (["neg"] if ABS_MODE == "dve" else []),
            "v": ["u"], "r": ["v"], "corr": ["r", "v"], "tsp": ["r", "corr"],
        }
        if ABS_MODE == "dve":
            need_after["neg"] = ["diff"]
        if not direct:
            need_after["o0"] = ["v"]
            need_after["o63"] = ["v"]
        for k, deps in need_after.items():
            for d in deps:
                assert po.index(k) > po.index(d), f"order: {k} before {d}"
    assert sum(CHUNK_COLS) == F
    assert len(RAMPS) == nchunks
    assert all(RAMPS[i] >= RAMPS[i + 1] for i in range(nchunks - 1))
    starts = [sum(CHUNK_COLS[:i]) for i in range(nchunks)]

    with tile.TileContext(nc) as tc:
        with (
            tc.tile_pool(name="prep", bufs=1) as prep,
            tc.tile_pool(name="pp", bufs=PP_BUFS) as pp,
            tc.tile_pool(name="outp", bufs=OUT_BUFS) as outp,
        ):
            pk_sb = prep.tile([P, NPAR + F], f32)
            v = prep.tile([P, FN], f32)
            tsp = prep.tile([P, FN], f32)

            if IN_SPLIT is None:
                nc.sync.dma_start(pk_sb[:], pk_d[:])
            else:
                cut = NPAR + IN_SPLIT
                nc.sync.dma_start(pk_sb[:, :cut], pk_d[:, :cut])
                nc.scalar.dma_start(pk_sb[:, cut:], pk_d[:, cut:])

            eng_map = {"sp": nc.sync, "act": nc.scalar, "pool": nc.gpsimd}
            eidx = {"early": [0], "steady": [0]}

            def dma(kind, dst, src):
                ring = EARLY_ENGINES if kind == "early" else STEADY_ENGINES
                i = eidx[kind]
                eng_map[ring[i[0] % len(ring)]].dma_start(dst, src)
                i[0] += 1

            def bcast(ap, w):
                return ap.rearrange("p (f n) -> p f n", f=1).broadcast_to([P, w, N])

            def onehot_piece(t, ws, wc, kind):
                o = outp.tile([P, FN], f32, tag="onehot")
                if t == 0:
                    nc.vector.tensor_scalar(
                        o[:, :wc], v[:, ws], 1.0, None, op0=AluOpType.is_lt
                    )
                elif t == T - 1:
                    nc.vector.tensor_scalar(
                        o[:, :wc], v[:, ws], float(T - 1), None,
                        op0=AluOpType.is_ge,
                    )
                else:
                    nc.vector.tensor_scalar(
                        o[:, :wc], tsp[:, ws], float(t), None,
                        op0=AluOpType.is_equal,
                    )
                dma(kind, out_d[t, :, ws], o[:, :wc])

            def direct_piece(t, ws, wc, x_b, thr_a, thr_b, sfx, kind):
                """o0: (x > L0)*(x < H0); o63: (x >= H63)+(x <= L63)."""
                w = wc // N
                a = pp.tile([P, wc], f32, tag=f"da{sfx}")
                b = pp.tile([P, wc], f32, tag=f"db{sfx}")
                o = outp.tile([P, FN], f32, tag="onehot")
                if t == 0:
                    op_a, op_b, combine = (
                        AluOpType.is_gt, AluOpType.is_lt, AluOpType.mult)
                else:
                    op_a, op_b, combine = (
                        AluOpType.is_ge, AluOpType.is_le, AluOpType.add)
                nc.vector.tensor_tensor(
                    a[:].rearrange("p (f n) -> p f n", n=N),
                    x_b, bcast(thr_a, w), op=op_a,
                )
                nc.vector.tensor_tensor(
                    b[:].rearrange("p (f n) -> p f n", n=N),
                    x_b, bcast(thr_b, w), op=op_b,
                )
                nc.vector.tensor_tensor(o[:, :wc], a[:], b[:], op=combine)
                dma(kind, out_d[t, :, ws], o[:, :wc])

            def prep_ops(c):
                col, w = starts[c], CHUNK_COLS[c]
                e0, e1 = col * N, (col + w) * N
                wc = w * N
                ws = slice(e0, e1)
                direct = c in HEAD_DIRECT
                x_b = pk_sb[:, NPAR + col : NPAR + col + w].broadcast_to(
                    [P, w, N]
                )
                cen_b = bcast(pk_sb[:, 0:N], w)
                sca_b = bcast(pk_sb[:, N : 2 * N], w)
                sfx = "" if SHARE_TAGS else str(c)
                diff = pp.tile([P, wc], f32, tag=f"diff{sfx}")
                neg = (pp.tile([P, wc], f32, tag=f"neg{sfx}")
                       if ABS_MODE == "dve" else None)
                u = pp.tile([P, wc], f32, tag=f"u{sfx}")
                r = pp.tile([P, wc], f32, tag=f"r{sfx}")
                corr = pp.tile([P, wc], f32, tag=f"corr{sfx}")
                ops = {
                    "diff": lambda: nc.vector.tensor_tensor(
                        diff[:].rearrange("p (f n) -> p f n", n=N),
                        x_b, cen_b, op=AluOpType.subtract,
                    ),
                    "neg": lambda: (
                        nc.vector.tensor_scalar(
                            neg[:], diff[:], -1.0, None, op0=AluOpType.mult
                        ) if ABS_MODE == "dve" else None
                    ),
                    "u": lambda: (
                        nc.vector.tensor_tensor(
                            u[:], diff[:], neg[:], op=AluOpType.max
                        ) if ABS_MODE == "dve" else
                        nc.scalar.activation(
                            u[:], diff[:], mybir.ActivationFunctionType.Abs
                        )
                    ),
                    "v": lambda: nc.vector.tensor_tensor(
                        v[:, ws].rearrange("p (f n) -> p f n", n=N),
                        u[:].rearrange("p (f n) -> p f n", n=N),
                        sca_b, op=AluOpType.mult,
                    ),
                    "o0": lambda: (
                        direct_piece(
                            0, ws, wc, x_b,
                            pk_sb[:, 2 * N : 3 * N], pk_sb[:, 3 * N : 4 * N],
                            sfx, "early",
                        ) if direct else onehot_piece(0, ws, wc, "early")
                    ),
                    "o63": lambda: (
                        direct_piece(
                            T - 1, ws, wc, x_b,
                            pk_sb[:, 5 * N : 6 * N], pk_sb[:, 4 * N : 5 * N],
                            sfx, "early",
                        ) if direct else onehot_piece(T - 1, ws, wc, "early")
                    ),
                    "r": lambda: nc.vector.tensor_scalar(
                        r[:], v[:, ws], 8388608.0, 8388608.0,
                        op0=AluOpType.add, op1=AluOpType.subtract,
                    ),
                    "corr": lambda: nc.vector.tensor_tensor(
                        corr[:], r[:], v[:, ws], op=AluOpType.is_gt
                    ),
                    "tsp": lambda: nc.vector.tensor_tensor(
                        tsp[:, ws], r[:], corr[:], op=AluOpType.subtract
                    ),
                }
                order = DIRECT_ORDER if direct else PREP_ORDER
                return [ops[k] for k in order]

            def ramp_ops(c):
                col, w = starts[c], CHUNK_COLS[c]
                e0, e1 = col * N, (col + w) * N
                ws = slice(e0, e1)
                return [
                    (lambda t=t: onehot_piece(t, ws, w * N, "early"))
                    for t in range(1, 1 + RAMPS[c])
                ]

            def emit_pass(use_hints):
                # chunk 0 prep, then ramps of chunk c interleaved with c+1 prep
                for op in prep_ops(0):
                    op()
                for c in range(nchunks):
                    ramps = ramp_ops(c)
                    nxt = prep_ops(c + 1) if c + 1 < nchunks else []
                    ri = 0
                    hint = (WAIT_HINTS[c + 1]
                            if use_hints and WAIT_HINTS and c + 1 < nchunks
                            else None)

                    def emit_next():
                        if hint is None:
                            nxt.pop(0)()
                        else:
                            with tc.tile_wait_until(hint):
                                nxt.pop(0)()

                    for op in ramps:
                        op()
                        ri += 1
                        if ri % INTERLEAVE == 0 and nxt:
                            emit_next()
                    while nxt:
                        emit_next()

                # complement pieces for ramp rows + full rows, ascending t
                for t in range(1 + RAMPS[-1], T - 1):
                    covered = sum(
                        CHUNK_COLS[c] for c in range(nchunks) if RAMPS[c] >= t
                    )
                    e0 = covered * N
                    if e0 >= FN:
                        continue
                    onehot_piece(t, slice(e0, FN), FN - e0, "steady")

            for rep in range(repeat):
                emit_pass(use_hints=(rep == 0))

    nc.compile()
    return nc


def _v_np(x, c, s):
    """Replicate the f32 v = s * |x - c| exactly as reference and device do."""
    d = np.abs(np.float32(np.float32(x) - np.float32(c)))
    return np.float32(np.float32(s) * d)


def _f2o(f):
    i = np.int64(np.float32(f).view(np.int32))
    return i if i >= 0 else -(i & 0x7FFFFFFF)


def _o2f(o):
    bits = np.uint32(o) if o >= 0 else np.uint32(0x80000000 | (-o))
    return bits.view(np.float32)


def _bisect_flip(pred, lo, hi):
    """Smallest f32 x in (lo, hi] with pred(x) True; pred monotone 0->1.
    lo has pred False, hi has pred True (ordinals)."""
    lo, hi = _f2o(lo), _f2o(hi)
    while hi - lo > 1:
        mid = (lo + hi) // 2
        if pred(_o2f(mid)):
            hi = mid
        else:
            lo = mid
    return _o2f(hi)


def _row_thresholds(cen, sca):
    """Per-n f32 thresholds: o0 = (x > L0) & (x < H0);
    o63 = (x >= H63) | (x <= L63). Exact by bisection of the reference
    predicate over the f32 lattice (monotone in x on each side of center)."""
    big = np.float32(3.0e38)
    L0 = np.empty(N, np.float32)
    H0 = np.empty(N, np.float32)
    L63 = np.empty(N, np.float32)
    H63 = np.empty(N, np.float32)
    for n in range(N):
        c, s = cen[n], sca[n]
        # t=0 fires iff v < 1; t=63 iff v >= 63 (v monotone in |x - c|).
        p0 = lambda x: _v_np(x, c, s) >= np.float32(1.0)
        p63 = lambda x: _v_np(x, c, s) >= np.float32(T - 1)
        # upper side: predicates go False -> True as x increases from c
        H0[n] = (_bisect_flip(p0, c, big)
                 if p0(big) else np.float32(np.inf))
        H63[n] = (_bisect_flip(p63, c, big)
                  if p63(big) else np.float32(np.inf))
        # lower side (mirror; the f32 lattice is sign-symmetric)
        L0[n] = (-_bisect_flip(lambda x: p0(-x), -c, big)
                 if p0(-big) else np.float32(-np.inf))
        L63[n] = (-_bisect_flip(lambda x: p63(-x), -c, big)
                  if p63(-big) else np.float32(-np.inf))
    # o0 is (x > L0) & (x < H0): H0/L0 are the first x past the t=0 region,
    # so flip to strict bounds: x < H0 keeps t=0; x > L0 likewise.
    return L0, H0, L63, H63


def kernel(x, center, scaling, time_steps):
    assert int(time_steps) == T
    x = np.ascontiguousarray(np.asarray(x, dtype=np.float32)).reshape(
        NCORES, P, F
    )
    cen = np.asarray(center, np.float32).reshape(N)
    sca = np.asarray(scaling, np.float32).reshape(N)
    NPAR = _npar()
    packed = np.empty((NCORES, P, NPAR + F), np.float32)
    packed[:, :, 0:N] = cen
    packed[:, :, N : 2 * N] = sca
    if HEAD_DIRECT:
        L0, H0, L63, H63 = _row_thresholds(cen, sca)
        packed[:, :, 2 * N : 3 * N] = L0
        packed[:, :, 3 * N : 4 * N] = H0
        packed[:, :, 4 * N : 5 * N] = L63
        packed[:, :, 5 * N : 6 * N] = H63
    packed[:, :, NPAR:] = x
    in_maps = [{"packed": packed[c]} for c in range(NCORES)]
    nc = _get_program()
    res = run_bass_kernel_spmd(nc, in_maps, core_ids=list(range(NCORES)))
    outs = [res.results[c]["out"].reshape(T, BC, N) for c in range(NCORES)]
    return np.concatenate(outs, axis=1)


def _get_program():
    if "nc" not in _cached:
        _cached["nc"] = build_program()
    return _cached["nc"]


# revision 30
# speedup vs baseline: 1.0007x; 1.0007x over previous
"""Trainium2 Bass kernel: receptive-field one-hot time encoder.

reference semantics:
    t_spike[n, b] = clip(int(scaling[n] * |x[b] - center[n]|), 0, T-1)
    out[t, b, n]  = 1.0 if t == t_spike[n, b] else 0.0       # [T, B, N] f32

Data-parallel over 8 cores: x and the output batch axis are sharded into 8
contiguous blocks of B_c = 16384; center/scaling are replicated.

Per-core layout: x viewed as [128 partitions, 128 cols]; all per-(b, n)
quantities live in [128, 2048] tiles with free index f*16+n, which makes each
output t-slice a single fully contiguous 1 MiB DRAM store ([T, B_c, N] with
b = p*128+f).

The kernel is DMA-write-bound: 64 MiB of output per core at 360 GB/s is
~186.4 us, so everything else is organized to keep the DMA engines saturated
from the earliest possible instant:

  - a small input DMA (params + x), split so the first chunk's x lands first;
  - an exact floor chain (DVE + one ACT hop; all ops HW-valid -- the DVE
    tensor_scalar has no mod/abs_max/floor on real silicon):
        diff = x - center                  (tensor_tensor, broadcast APs)
        u    = |diff|                      (ACT Abs)
        v    = u * scaling                 (tensor_tensor)
        r    = (v + 2^23) - 2^23           (dual-op tensor_scalar: nearest int)
        tsp  = r - (r > v)                 (exact floor for 0 <= v < 2^23 under
                                            any nearest rounding of the add)
    one-hot rows: t=0 is_lt(v,1), t=63 is_ge(v,63), inner is_equal(tsp,t);
    (HEAD_DIRECT can instead compare x against host-bisected per-n
    thresholds for t=0/63 -- exact but 3 ops, off by default: the extra DVE
    serial time delays the ramp rows more than the head gains);
  - x-columns processed in a ramp of growing chunks; chunk c emits one-hot
    row pieces for t = 0, 63, 1..RAMPS[c] over its own columns, with the next
    chunk's prep ops interleaved between emissions (plus tile-scheduler wait
    hints) so the DMA backlog never drains during prep bursts;
  - rows RAMPS[c] < t <= RAMPS[c-1] get a single complement piece covering
    the later chunks' columns once the full tsp tile exists, then rows
    t > RAMPS[0] go out as single full-width DMAs;
  - early DMAs alternate between the SP/Act HWDGE paths and the Pool SWDGE
    path so descriptor generation never starves the DMA engines.

All compares are exact on the f32 lattice, so outputs match the jax reference
bit-for-bit (relative error 0.0, verified on HW).
"""

import sys

if "/opt/trn_rl_repo" not in sys.path:
    sys.path.insert(0, "/opt/trn_rl_repo")

import numpy as np

import concourse.tile as tile
from concourse import bacc, mybir
from concourse.alu_op_type import AluOpType
from concourse.bass_utils import run_bass_kernel_spmd

B, N, T = 131072, 16, 64
NCORES = 8
BC = B // NCORES  # 16384 per core
P = 128
F = BC // P  # 128 x-columns per partition
FN = F * N  # 2048 free elems per output row
HEAD_DIRECT = []                # chunks whose o0/o63 compare x directly


def _npar():
    return 6 * N if HEAD_DIRECT else 2 * N

# --- tunables -------------------------------------------------------------
CHUNK_COLS = [16, 24, 32, 56]   # x-column ramp (must sum to F)
RAMPS = [16, 16, 6, 2]          # per-chunk ramp rows t=1..RAMPS[c] (decreasing)
INTERLEAVE = 2                  # ramp emissions between interleaved prep ops
OUT_BUFS = 16
IN_SPLIT = 16                   # None = one input DMA; else split after this many x cols
SHARE_TAGS = True               # share prep-tile tags across chunks (WAR serialization)
PP_BUFS = 1                     # ring depth of the prep-tile pool
WAIT_HINTS = [None, 0.005, 0.009, 0.013]  # per-chunk scheduler wait hint (ms)
PREP_ORDER = ["diff", "u", "v", "r", "o0", "corr", "o63", "tsp"]
DIRECT_ORDER = ["o0", "o63", "diff", "u", "v", "r", "corr", "tsp"]
ABS_MODE = "act"                # "dve": max(diff,-diff); "act": ACT Abs
PAIR_T = 2                      # adjacent t-rows per early ramp DMA
EARLY_ENGINES = ["pool", "sp", "act"]   # issue ring for chunk-piece DMAs
STEADY_ENGINES = ["sp", "act"]          # issue ring for complement/full rows

f32 = mybir.dt.float32

_cached = {}


def build_program(repeat=1):
    nc = bacc.Bacc(
        "TRN2", target_bir_lowering=False, debug=False, enable_asserts=False
    )
    # packed: [:, :N]=center, [:, N:2N]=scaling, [:, 2N:6N]=L0,H0,L63,H63,
    # [:, 6N:]=x
    NPAR = _npar()
    pk_d = nc.dram_tensor("packed", [P, NPAR + F], f32, kind="ExternalInput").ap()
    out_d = nc.dram_tensor("out", [T, P, FN], f32, kind="ExternalOutput").ap()

    nchunks = len(CHUNK_COLS)
    for order, direct in ((PREP_ORDER, False), (DIRECT_ORDER, True)):
        po = list(order)
        need_after = {
            "u": ["diff"] + (["neg"] if ABS_MODE == "dve" else []),
            "v": ["u"], "r": ["v"], "corr": ["r", "v"], "tsp": ["r", "corr"],
        }
        if ABS_MODE == "dve":
            need_after["neg"] = ["diff"]
        if not direct:
            need_after["o0"] = ["v"]
            need_after["o63"] = ["v"]
        for k, deps in need_after.items():
            for d in deps:
                assert po.index(k) > po.index(d), f"order: {k} before {d}"
    assert sum(CHUNK_COLS) == F
    assert len(RAMPS) == nchunks
    assert all(RAMPS[i] >= RAMPS[i + 1] for i in range(nchunks - 1))
    starts = [sum(CHUNK_COLS[:i]) for i in range(nchunks)]

    with tile.TileContext(nc) as tc:
        with (
            tc.tile_pool(name="prep", bufs=1) as prep,
            tc.tile_pool(name="pp", bufs=PP_BUFS) as pp,
            tc.tile_pool(name="outp", bufs=OUT_BUFS) as outp,
        ):
            pk_sb = prep.tile([P, NPAR + F], f32)
            v = prep.tile([P, FN], f32)
            tsp = prep.tile([P, FN], f32)

            if IN_SPLIT is None:
                nc.sync.dma_start(pk_sb[:], pk_d[:])
            else:
                cut = NPAR + IN_SPLIT
                nc.sync.dma_start(pk_sb[:, :cut], pk_d[:, :cut])
                nc.scalar.dma_start(pk_sb[:, cut:], pk_d[:, cut:])

            eng_map = {"sp": nc.sync, "act": nc.scalar, "pool": nc.gpsimd}
            eidx = {"early": [0], "steady": [0]}

            def dma(kind, dst, src):
                ring = EARLY_ENGINES if kind == "early" else STEADY_ENGINES
                i = eidx[kind]
                eng_map[ring[i[0] % len(ring)]].dma_start(dst, src)
                i[0] += 1

            def bcast(ap, w):
                return ap.rearrange("p (f n) -> p f n", f=1).broadcast_to([P, w, N])

            def onehot_piece(t, ws, wc, kind):
                o = outp.tile([P, FN], f32, tag="onehot")
                if t == 0:
                    nc.vector.tensor_scalar(
                        o[:, :wc], v[:, ws], 1.0, None, op0=AluOpType.is_lt
                    )
                elif t == T - 1:
                    nc.vector.tensor_scalar(
                        o[:, :wc], v[:, ws], float(T - 1), None,
                        op0=AluOpType.is_ge,
                    )
                else:
                    nc.vector.tensor_scalar(
                        o[:, :wc], tsp[:, ws], float(t), None,
                        op0=AluOpType.is_equal,
                    )
                dma(kind, out_d[t, :, ws], o[:, :wc])

            def direct_piece(t, ws, wc, x_b, thr_a, thr_b, sfx, kind):
                """o0: (x > L0)*(x < H0); o63: (x >= H63)+(x <= L63)."""
                w = wc // N
                a = pp.tile([P, wc], f32, tag=f"da{sfx}")
                b = pp.tile([P, wc], f32, tag=f"db{sfx}")
                o = outp.tile([P, FN], f32, tag="onehot")
                if t == 0:
                    op_a, op_b, combine = (
                        AluOpType.is_gt, AluOpType.is_lt, AluOpType.mult)
                else:
                    op_a, op_b, combine = (
                        AluOpType.is_ge, AluOpType.is_le, AluOpType.add)
                nc.vector.tensor_tensor(
                    a[:].rearrange("p (f n) -> p f n", n=N),
                    x_b, bcast(thr_a, w), op=op_a,
                )
                nc.vector.tensor_tensor(
                    b[:].rearrange("p (f n) -> p f n", n=N),
                    x_b, bcast(thr_b, w), op=op_b,
                )
                nc.vector.tensor_tensor(o[:, :wc], a[:], b[:], op=combine)
                dma(kind, out_d[t, :, ws], o[:, :wc])

            def prep_ops(c):
                col, w = starts[c], CHUNK_COLS[c]
                e0, e1 = col * N, (col + w) * N
                wc = w * N
                ws = slice(e0, e1)
                direct = c in HEAD_DIRECT
                x_b = pk_sb[:, NPAR + col : NPAR + col + w].broadcast_to(
                    [P, w, N]
                )
                cen_b = bcast(pk_sb[:, 0:N], w)
                sca_b = bcast(pk_sb[:, N : 2 * N], w)
                sfx = "" if SHARE_TAGS else str(c)
                diff = pp.tile([P, wc], f32, tag=f"diff{sfx}")
                neg = (pp.tile([P, wc], f32, tag=f"neg{sfx}")
                       if ABS_MODE == "dve" else None)
                u = pp.tile([P, wc], f32, tag=f"u{sfx}")
                r = pp.tile([P, wc], f32, tag=f"r{sfx}")
                corr = pp.tile([P, wc], f32, tag=f"corr{sfx}")
                ops = {
                    "diff": lambda: nc.vector.tensor_tensor(
                        diff[:].rearrange("p (f n) -> p f n", n=N),
                        x_b, cen_b, op=AluOpType.subtract,
                    ),
                    "neg": lambda: (
                        nc.vector.tensor_scalar(
                            neg[:], diff[:], -1.0, None, op0=AluOpType.mult
                        ) if ABS_MODE == "dve" else None
                    ),
                    "u": lambda: (
                        nc.vector.tensor_tensor(
                            u[:], diff[:], neg[:], op=AluOpType.max
                        ) if ABS_MODE == "dve" else
                        nc.scalar.activation(
                            u[:], diff[:], mybir.ActivationFunctionType.Abs
                        )
                    ),
                    "v": lambda: nc.vector.tensor_tensor(
                        v[:, ws].rearrange("p (f n) -> p f n", n=N),
                        u[:].rearrange("p (f n) -> p f n", n=N),
                        sca_b, op=AluOpType.mult,
                    ),
                    "o0": lambda: (
                        direct_piece(
                            0, ws, wc, x_b,
                            pk_sb[:, 2 * N : 3 * N], pk_sb[:, 3 * N : 4 * N],
                            sfx, "early",
                        ) if direct else onehot_piece(0, ws, wc, "early")
                    ),
                    "o63": lambda: (
                        direct_piece(
                            T - 1, ws, wc, x_b,
                            pk_sb[:, 5 * N : 6 * N], pk_sb[:, 4 * N : 5 * N],
                            sfx, "early",
                        ) if direct else onehot_piece(T - 1, ws, wc, "early")
                    ),
                    "r": lambda: nc.vector.tensor_scalar(
                        r[:], v[:, ws], 8388608.0, 8388608.0,
                        op0=AluOpType.add, op1=AluOpType.subtract,
                    ),
                    "corr": lambda: nc.vector.tensor_tensor(
                        corr[:], r[:], v[:, ws], op=AluOpType.is_gt
                    ),
                    "tsp": lambda: nc.vector.tensor_tensor(
                        tsp[:, ws], r[:], corr[:], op=AluOpType.subtract
                    ),
                }
                order = DIRECT_ORDER if direct else PREP_ORDER
                return [ops[k] for k in order]

            def ramp_pair(t0_, tcnt, ws, wc, kind):
                """tcnt adjacent t-rows in ONE DMA: descriptor-gen cost is
                per-instruction, so a strided [tcnt, P, wc] DRAM AP moves
                multiple rows for one HWDGE/SWDGE slot."""
                o = outp.tile([P, FN], f32, tag="onehot")
                for k in range(tcnt):
                    nc.vector.tensor_scalar(
                        o[:, k * wc : (k + 1) * wc], tsp[:, ws],
                        float(t0_ + k), None, op0=AluOpType.is_equal,
                    )
                dma(
                    kind,
                    out_d[t0_ : t0_ + tcnt, :, ws].rearrange("k p w -> p k w"),
                    o[:, : tcnt * wc].rearrange("p (k w) -> p k w", k=tcnt),
                )

            def ramp_ops(c):
                col, w = starts[c], CHUNK_COLS[c]
                e0, e1 = col * N, (col + w) * N
                ws = slice(e0, e1)
                wc = w * N
                ops = []
                t = 1
                while t <= RAMPS[c]:
                    tcnt = min(PAIR_T, RAMPS[c] - t + 1, FN // wc)
                    ops.append(
                        lambda t=t, tcnt=tcnt: ramp_pair(t, tcnt, ws, wc, "early")
                    )
                    t += tcnt
                return ops

            def emit_pass(use_hints):
                # chunk 0 prep, then ramps of chunk c interleaved with c+1 prep
                for op in prep_ops(0):
                    op()
                for c in range(nchunks):
                    ramps = ramp_ops(c)
                    nxt = prep_ops(c + 1) if c + 1 < nchunks else []
                    ri = 0
                    hint = (WAIT_HINTS[c + 1]
                            if use_hints and WAIT_HINTS and c + 1 < nchunks
                            else None)

                    def emit_next():
                        if hint is None:
                            nxt.pop(0)()
                        else:
                            with tc.tile_wait_until(hint):
                                nxt.pop(0)()

                    for op in ramps:
                        op()
                        ri += 1
                        if ri % INTERLEAVE == 0 and nxt:
                            emit_next()
                    while nxt:
                        emit_next()

                # complement pieces for ramp rows + full rows, ascending t
                for t in range(1 + RAMPS[-1], T - 1):
                    covered = sum(
                        CHUNK_COLS[c] for c in range(nchunks) if RAMPS[c] >= t
                    )
                    e0 = covered * N
                    if e0 >= FN:
                        continue
                    onehot_piece(t, slice(e0, FN), FN - e0, "steady")

            for rep in range(repeat):
                emit_pass(use_hints=(rep == 0))

    nc.compile()
    return nc


def _v_np(x, c, s):
    """Replicate the f32 v = s * |x - c| exactly as reference and device do."""
    d = np.abs(np.float32(np.float32(x) - np.float32(c)))
    return np.float32(np.float32(s) * d)


def _f2o(f):
    i = np.int64(np.float32(f).view(np.int32))
    return i if i >= 0 else -(i & 0x7FFFFFFF)


def _o2f(o):
    bits = np.uint32(o) if o >= 0 else np.uint32(0x80000000 | (-o))
    return bits.view(np.float32)


def _bisect_flip(pred, lo, hi):
    """Smallest f32 x in (lo, hi] with pred(x) True; pred monotone 0->1.
    lo has pred False, hi has pred True (ordinals)."""
    lo, hi = _f2o(lo), _f2o(hi)
    while hi - lo > 1:
        mid = (lo + hi) // 2
        if pred(_o2f(mid)):
            hi = mid
        else:
            lo = mid
    return _o2f(hi)


def _row_thresholds(cen, sca):
    """Per-n f32 thresholds: o0 = (x > L0) & (x < H0);
    o63 = (x >= H63) | (x <= L63). Exact by bisection of the reference
    predicate over the f32 lattice (monotone in x on each side of center)."""
    big = np.float32(3.0e38)
    L0 = np.empty(N, np.float32)
    H0 = np.empty(N, np.float32)
    L63 = np.empty(N, np.float32)
    H63 = np.empty(N, np.float32)
    for n in range(N):
        c, s = cen[n], sca[n]
        # t=0 fires iff v < 1; t=63 iff v >= 63 (v monotone in |x - c|).
        p0 = lambda x: _v_np(x, c, s) >= np.float32(1.0)
        p63 = lambda x: _v_np(x, c, s) >= np.float32(T - 1)
        # upper side: predicates go False -> True as x increases from c
        H0[n] = (_bisect_flip(p0, c, big)
                 if p0(big) else np.float32(np.inf))
        H63[n] = (_bisect_flip(p63, c, big)
                  if p63(big) else np.float32(np.inf))
        # lower side (mirror; the f32 lattice is sign-symmetric)
        L0[n] = (-_bisect_flip(lambda x: p0(-x), -c, big)
                 if p0(-big) else np.float32(-np.inf))
        L63[n] = (-_bisect_flip(lambda x: p63(-x), -c, big)
                  if p63(-big) else np.float32(-np.inf))
    # o0 is (x > L0) & (x < H0): H0/L0 are the first x past the t=0 region,
    # so flip to strict bounds: x < H0 keeps t=0; x > L0 likewise.
    return L0, H0, L63, H63


def kernel(x, center, scaling, time_steps):
    assert int(time_steps) == T
    x = np.ascontiguousarray(np.asarray(x, dtype=np.float32)).reshape(
        NCORES, P, F
    )
    cen = np.asarray(center, np.float32).reshape(N)
    sca = np.asarray(scaling, np.float32).reshape(N)
    NPAR = _npar()
    packed = np.empty((NCORES, P, NPAR + F), np.float32)
    packed[:, :, 0:N] = cen
    packed[:, :, N : 2 * N] = sca
    if HEAD_DIRECT:
        L0, H0, L63, H63 = _row_thresholds(cen, sca)
        packed[:, :, 2 * N : 3 * N] = L0
        packed[:, :, 3 * N : 4 * N] = H0
        packed[:, :, 4 * N : 5 * N] = L63
        packed[:, :, 5 * N : 6 * N] = H63
    packed[:, :, NPAR:] = x
    in_maps = [{"packed": packed[c]} for c in range(NCORES)]
    nc = _get_program()
    res = run_bass_kernel_spmd(nc, in_maps, core_ids=list(range(NCORES)))
    outs = [res.results[c]["out"].reshape(T, BC, N) for c in range(NCORES)]
    return np.concatenate(outs, axis=1)


def _get_program():
    if "nc" not in _cached:
        _cached["nc"] = build_program()
    return _cached["nc"]


# revision 31
# speedup vs baseline: 1.0023x; 1.0016x over previous
"""Trainium2 Bass kernel: receptive-field one-hot time encoder.

reference semantics:
    t_spike[n, b] = clip(int(scaling[n] * |x[b] - center[n]|), 0, T-1)
    out[t, b, n]  = 1.0 if t == t_spike[n, b] else 0.0       # [T, B, N] f32

Data-parallel over 8 cores: x and the output batch axis are sharded into 8
contiguous blocks of B_c = 16384; center/scaling are replicated.

Per-core layout: x viewed as [128 partitions, 128 cols]; all per-(b, n)
quantities live in [128, 2048] tiles with free index f*16+n, which makes each
output t-slice a single fully contiguous 1 MiB DRAM store ([T, B_c, N] with
b = p*128+f).

The kernel is DMA-write-bound: 64 MiB of output per core at 360 GB/s is
~186.4 us, so everything else is organized to keep the DMA engines saturated
from the earliest possible instant:

  - a small input DMA (params + x), split so the first chunk's x lands first;
  - an exact floor chain (DVE + one ACT hop; all ops HW-valid -- the DVE
    tensor_scalar has no mod/abs_max/floor on real silicon):
        diff = x - center                  (tensor_tensor, broadcast APs)
        u    = |diff|                      (ACT Abs)
        v    = u * scaling                 (tensor_tensor)
        r    = (v + 2^23) - 2^23           (dual-op tensor_scalar: nearest int)
        tsp  = r - (r > v)                 (exact floor for 0 <= v < 2^23 under
                                            any nearest rounding of the add)
    one-hot rows: t=0 is_lt(v,1), t=63 is_ge(v,63), inner is_equal(tsp,t);
    (HEAD_DIRECT can instead compare x against host-bisected per-n
    thresholds for t=0/63 -- exact but 3 ops, off by default: the extra DVE
    serial time delays the ramp rows more than the head gains);
  - x-columns processed in a ramp of growing chunks; chunk c emits one-hot
    row pieces for t = 0, 63, 1..RAMPS[c] over its own columns, with the next
    chunk's prep ops interleaved between emissions (plus tile-scheduler wait
    hints) so the DMA backlog never drains during prep bursts;
  - rows RAMPS[c] < t <= RAMPS[c-1] get a single complement piece covering
    the later chunks' columns once the full tsp tile exists, then rows
    t > RAMPS[0] go out as single full-width DMAs;
  - early DMAs alternate between the SP/Act HWDGE paths and the Pool SWDGE
    path so descriptor generation never starves the DMA engines.

All compares are exact on the f32 lattice, so outputs match the jax reference
bit-for-bit (relative error 0.0, verified on HW).
"""

import sys

if "/opt/trn_rl_repo" not in sys.path:
    sys.path.insert(0, "/opt/trn_rl_repo")

import numpy as np

import concourse.tile as tile
from concourse import bacc, mybir
from concourse.alu_op_type import AluOpType
from concourse.bass_utils import run_bass_kernel_spmd

B, N, T = 131072, 16, 64
NCORES = 8
BC = B // NCORES  # 16384 per core
P = 128
F = BC // P  # 128 x-columns per partition
FN = F * N  # 2048 free elems per output row
HEAD_DIRECT = []                # chunks whose o0/o63 compare x directly


def _npar():
    return 6 * N if HEAD_DIRECT else 2 * N

# --- tunables -------------------------------------------------------------
CHUNK_COLS = [12, 24, 32, 60]   # x-column ramp (must sum to F)
RAMPS = [18, 14, 6, 2]          # per-chunk ramp rows t=1..RAMPS[c] (decreasing)
INTERLEAVE = 2                  # ramp emissions between interleaved prep ops
OUT_BUFS = 16
IN_SPLIT = 12                   # None = one input DMA; else split after this many x cols
SHARE_TAGS = True               # share prep-tile tags across chunks (WAR serialization)
PP_BUFS = 1                     # ring depth of the prep-tile pool
WAIT_HINTS = [None, 0.005, 0.009, 0.013]  # per-chunk scheduler wait hint (ms)
PREP_ORDER = ["diff", "u", "v", "r", "o0", "corr", "o63", "tsp"]
DIRECT_ORDER = ["o0", "o63", "diff", "u", "v", "r", "corr", "tsp"]
ABS_MODE = "act"                # "dve": max(diff,-diff); "act": ACT Abs
PAIR_T = 2                      # adjacent t-rows per early ramp DMA
EARLY_ENGINES = ["pool", "sp", "act"]   # issue ring for chunk-piece DMAs
STEADY_ENGINES = ["sp", "act"]          # issue ring for complement/full rows

f32 = mybir.dt.float32

_cached = {}


def build_program(repeat=1):
    nc = bacc.Bacc(
        "TRN2", target_bir_lowering=False, debug=False, enable_asserts=False
    )
    # packed: [:, :N]=center, [:, N:2N]=scaling, [:, 2N:6N]=L0,H0,L63,H63,
    # [:, 6N:]=x
    NPAR = _npar()
    pk_d = nc.dram_tensor("packed", [P, NPAR + F], f32, kind="ExternalInput").ap()
    out_d = nc.dram_tensor("out", [T, P, FN], f32, kind="ExternalOutput").ap()

    nchunks = len(CHUNK_COLS)
    for order, direct in ((PREP_ORDER, False), (DIRECT_ORDER, True)):
        po = list(order)
        need_after = {
            "u": ["diff"] + (["neg"] if ABS_MODE == "dve" else []),
            "v": ["u"], "r": ["v"], "corr": ["r", "v"], "tsp": ["r", "corr"],
        }
        if ABS_MODE == "dve":
            need_after["neg"] = ["diff"]
        if not direct:
            need_after["o0"] = ["v"]
            need_after["o63"] = ["v"]
        for k, deps in need_after.items():
            for d in deps:
                assert po.index(k) > po.index(d), f"order: {k} before {d}"
    assert sum(CHUNK_COLS) == F
    assert len(RAMPS) == nchunks
    assert all(RAMPS[i] >= RAMPS[i + 1] for i in range(nchunks - 1))
    starts = [sum(CHUNK_COLS[:i]) for i in range(nchunks)]

    with tile.TileContext(nc) as tc:
        with (
            tc.tile_pool(name="prep", bufs=1) as prep,
            tc.tile_pool(name="pp", bufs=PP_BUFS) as pp,
            tc.tile_pool(name="outp", bufs=OUT_BUFS) as outp,
        ):
            pk_sb = prep.tile([P, NPAR + F], f32)
            v = prep.tile([P, FN], f32)
            tsp = prep.tile([P, FN], f32)

            if IN_SPLIT is None:
                nc.sync.dma_start(pk_sb[:], pk_d[:])
            else:
                cut = NPAR + IN_SPLIT
                nc.sync.dma_start(pk_sb[:, :cut], pk_d[:, :cut])
                nc.scalar.dma_start(pk_sb[:, cut:], pk_d[:, cut:])

            eng_map = {"sp": nc.sync, "act": nc.scalar, "pool": nc.gpsimd}
            eidx = {"early": [0], "steady": [0]}

            def dma(kind, dst, src):
                ring = EARLY_ENGINES if kind == "early" else STEADY_ENGINES
                i = eidx[kind]
                eng_map[ring[i[0] % len(ring)]].dma_start(dst, src)
                i[0] += 1

            def bcast(ap, w):
                return ap.rearrange("p (f n) -> p f n", f=1).broadcast_to([P, w, N])

            def onehot_piece(t, ws, wc, kind):
                o = outp.tile([P, FN], f32, tag="onehot")
                if t == 0:
                    nc.vector.tensor_scalar(
                        o[:, :wc], v[:, ws], 1.0, None, op0=AluOpType.is_lt
                    )
                elif t == T - 1:
                    nc.vector.tensor_scalar(
                        o[:, :wc], v[:, ws], float(T - 1), None,
                        op0=AluOpType.is_ge,
                    )
                else:
                    nc.vector.tensor_scalar(
                        o[:, :wc], tsp[:, ws], float(t), None,
                        op0=AluOpType.is_equal,
                    )
                dma(kind, out_d[t, :, ws], o[:, :wc])

            def direct_piece(t, ws, wc, x_b, thr_a, thr_b, sfx, kind):
                """o0: (x > L0)*(x < H0); o63: (x >= H63)+(x <= L63)."""
                w = wc // N
                a = pp.tile([P, wc], f32, tag=f"da{sfx}")
                b = pp.tile([P, wc], f32, tag=f"db{sfx}")
                o = outp.tile([P, FN], f32, tag="onehot")
                if t == 0:
                    op_a, op_b, combine = (
                        AluOpType.is_gt, AluOpType.is_lt, AluOpType.mult)
                else:
                    op_a, op_b, combine = (
                        AluOpType.is_ge, AluOpType.is_le, AluOpType.add)
                nc.vector.tensor_tensor(
                    a[:].rearrange("p (f n) -> p f n", n=N),
                    x_b, bcast(thr_a, w), op=op_a,
                )
                nc.vector.tensor_tensor(
                    b[:].rearrange("p (f n) -> p f n", n=N),
                    x_b, bcast(thr_b, w), op=op_b,
                )
                nc.vector.tensor_tensor(o[:, :wc], a[:], b[:], op=combine)
                dma(kind, out_d[t, :, ws], o[:, :wc])

            def prep_ops(c):
                col, w = starts[c], CHUNK_COLS[c]
                e0, e1 = col * N, (col + w) * N
                wc = w * N
                ws = slice(e0, e1)
                direct = c in HEAD_DIRECT
                x_b = pk_sb[:, NPAR + col : NPAR + col + w].broadcast_to(
                    [P, w, N]
                )
                cen_b = bcast(pk_sb[:, 0:N], w)
                sca_b = bcast(pk_sb[:, N : 2 * N], w)
                sfx = "" if SHARE_TAGS else str(c)
                diff = pp.tile([P, wc], f32, tag=f"diff{sfx}")
                neg = (pp.tile([P, wc], f32, tag=f"neg{sfx}")
                       if ABS_MODE == "dve" else None)
                u = pp.tile([P, wc], f32, tag=f"u{sfx}")
                r = pp.tile([P, wc], f32, tag=f"r{sfx}")
                corr = pp.tile([P, wc], f32, tag=f"corr{sfx}")
                ops = {
                    "diff": lambda: nc.vector.tensor_tensor(
                        diff[:].rearrange("p (f n) -> p f n", n=N),
                        x_b, cen_b, op=AluOpType.subtract,
                    ),
                    "neg": lambda: (
                        nc.vector.tensor_scalar(
                            neg[:], diff[:], -1.0, None, op0=AluOpType.mult
                        ) if ABS_MODE == "dve" else None
                    ),
                    "u": lambda: (
                        nc.vector.tensor_tensor(
                            u[:], diff[:], neg[:], op=AluOpType.max
                        ) if ABS_MODE == "dve" else
                        nc.scalar.activation(
                            u[:], diff[:], mybir.ActivationFunctionType.Abs
                        )
                    ),
                    "v": lambda: nc.vector.tensor_tensor(
                        v[:, ws].rearrange("p (f n) -> p f n", n=N),
                        u[:].rearrange("p (f n) -> p f n", n=N),
                        sca_b, op=AluOpType.mult,
                    ),
                    "o0": lambda: (
                        direct_piece(
                            0, ws, wc, x_b,
                            pk_sb[:, 2 * N : 3 * N], pk_sb[:, 3 * N : 4 * N],
                            sfx, "early",
                        ) if direct else onehot_piece(0, ws, wc, "early")
                    ),
                    "o63": lambda: (
                        direct_piece(
                            T - 1, ws, wc, x_b,
                            pk_sb[:, 5 * N : 6 * N], pk_sb[:, 4 * N : 5 * N],
                            sfx, "early",
                        ) if direct else onehot_piece(T - 1, ws, wc, "early")
                    ),
                    "r": lambda: nc.vector.tensor_scalar(
                        r[:], v[:, ws], 8388608.0, 8388608.0,
                        op0=AluOpType.add, op1=AluOpType.subtract,
                    ),
                    "corr": lambda: nc.vector.tensor_tensor(
                        corr[:], r[:], v[:, ws], op=AluOpType.is_gt
                    ),
                    "tsp": lambda: nc.vector.tensor_tensor(
                        tsp[:, ws], r[:], corr[:], op=AluOpType.subtract
                    ),
                }
                order = DIRECT_ORDER if direct else PREP_ORDER
                return [ops[k] for k in order]

            def ramp_pair(t0_, tcnt, ws, wc, kind):
                """tcnt adjacent t-rows in ONE DMA: descriptor-gen cost is
                per-instruction, so a strided [tcnt, P, wc] DRAM AP moves
                multiple rows for one HWDGE/SWDGE slot."""
                o = outp.tile([P, FN], f32, tag="onehot")
                for k in range(tcnt):
                    nc.vector.tensor_scalar(
                        o[:, k * wc : (k + 1) * wc], tsp[:, ws],
                        float(t0_ + k), None, op0=AluOpType.is_equal,
                    )
                dma(
                    kind,
                    out_d[t0_ : t0_ + tcnt, :, ws].rearrange("k p w -> p k w"),
                    o[:, : tcnt * wc].rearrange("p (k w) -> p k w", k=tcnt),
                )

            def ramp_ops(c):
                col, w = starts[c], CHUNK_COLS[c]
                e0, e1 = col * N, (col + w) * N
                ws = slice(e0, e1)
                wc = w * N
                ops = []
                t = 1
                while t <= RAMPS[c]:
                    tcnt = min(PAIR_T, RAMPS[c] - t + 1, FN // wc)
                    ops.append(
                        lambda t=t, tcnt=tcnt: ramp_pair(t, tcnt, ws, wc, "early")
                    )
                    t += tcnt
                return ops

            def emit_pass(use_hints):
                # chunk 0 prep, then ramps of chunk c interleaved with c+1 prep
                for op in prep_ops(0):
                    op()
                for c in range(nchunks):
                    ramps = ramp_ops(c)
                    nxt = prep_ops(c + 1) if c + 1 < nchunks else []
                    ri = 0
                    hint = (WAIT_HINTS[c + 1]
                            if use_hints and WAIT_HINTS and c + 1 < nchunks
                            else None)

                    def emit_next():
                        if hint is None:
                            nxt.pop(0)()
                        else:
                            with tc.tile_wait_until(hint):
                                nxt.pop(0)()

                    for op in ramps:
                        op()
                        ri += 1
                        if ri % INTERLEAVE == 0 and nxt:
                            emit_next()
                    while nxt:
                        emit_next()

                # complement pieces for ramp rows + full rows, ascending t
                for t in range(1 + RAMPS[-1], T - 1):
                    covered = sum(
                        CHUNK_COLS[c] for c in range(nchunks) if RAMPS[c] >= t
                    )
                    e0 = covered * N
                    if e0 >= FN:
                        continue
                    onehot_piece(t, slice(e0, FN), FN - e0, "steady")

            for rep in range(repeat):
                emit_pass(use_hints=(rep == 0))

    nc.compile()
    return nc


def _v_np(x, c, s):
    """Replicate the f32 v = s * |x - c| exactly as reference and device do."""
    d = np.abs(np.float32(np.float32(x) - np.float32(c)))
    return np.float32(np.float32(s) * d)


def _f2o(f):
    i = np.int64(np.float32(f).view(np.int32))
    return i if i >= 0 else -(i & 0x7FFFFFFF)


def _o2f(o):
    bits = np.uint32(o) if o >= 0 else np.uint32(0x80000000 | (-o))
    return bits.view(np.float32)


def _bisect_flip(pred, lo, hi):
    """Smallest f32 x in (lo, hi] with pred(x) True; pred monotone 0->1.
    lo has pred False, hi has pred True (ordinals)."""
    lo, hi = _f2o(lo), _f2o(hi)
    while hi - lo > 1:
        mid = (lo + hi) // 2
        if pred(_o2f(mid)):
            hi = mid
        else:
            lo = mid
    return _o2f(hi)


def _row_thresholds(cen, sca):
    """Per-n f32 thresholds: o0 = (x > L0) & (x < H0);
    o63 = (x >= H63) | (x <= L63). Exact by bisection of the reference
    predicate over the f32 lattice (monotone in x on each side of center)."""
    big = np.float32(3.0e38)
    L0 = np.empty(N, np.float32)
    H0 = np.empty(N, np.float32)
    L63 = np.empty(N, np.float32)
    H63 = np.empty(N, np.float32)
    for n in range(N):
        c, s = cen[n], sca[n]
        # t=0 fires iff v < 1; t=63 iff v >= 63 (v monotone in |x - c|).
        p0 = lambda x: _v_np(x, c, s) >= np.float32(1.0)
        p63 = lambda x: _v_np(x, c, s) >= np.float32(T - 1)
        # upper side: predicates go False -> True as x increases from c
        H0[n] = (_bisect_flip(p0, c, big)
                 if p0(big) else np.float32(np.inf))
        H63[n] = (_bisect_flip(p63, c, big)
                  if p63(big) else np.float32(np.inf))
        # lower side (mirror; the f32 lattice is sign-symmetric)
        L0[n] = (-_bisect_flip(lambda x: p0(-x), -c, big)
                 if p0(-big) else np.float32(-np.inf))
        L63[n] = (-_bisect_flip(lambda x: p63(-x), -c, big)
                  if p63(-big) else np.float32(-np.inf))
    # o0 is (x > L0) & (x < H0): H0/L0 are the first x past the t=0 region,
    # so flip to strict bounds: x < H0 keeps t=0; x > L0 likewise.
    return L0, H0, L63, H63


def kernel(x, center, scaling, time_steps):
    assert int(time_steps) == T
    x = np.ascontiguousarray(np.asarray(x, dtype=np.float32)).reshape(
        NCORES, P, F
    )
    cen = np.asarray(center, np.float32).reshape(N)
    sca = np.asarray(scaling, np.float32).reshape(N)
    NPAR = _npar()
    packed = np.empty((NCORES, P, NPAR + F), np.float32)
    packed[:, :, 0:N] = cen
    packed[:, :, N : 2 * N] = sca
    if HEAD_DIRECT:
        L0, H0, L63, H63 = _row_thresholds(cen, sca)
        packed[:, :, 2 * N : 3 * N] = L0
        packed[:, :, 3 * N : 4 * N] = H0
        packed[:, :, 4 * N : 5 * N] = L63
        packed[:, :, 5 * N : 6 * N] = H63
    packed[:, :, NPAR:] = x
    in_maps = [{"packed": packed[c]} for c in range(NCORES)]
    nc = _get_program()
    res = run_bass_kernel_spmd(nc, in_maps, core_ids=list(range(NCORES)))
    outs = [res.results[c]["out"].reshape(T, BC, N) for c in range(NCORES)]
    return np.concatenate(outs, axis=1)


def _get_program():
    if "nc" not in _cached:
        _cached["nc"] = build_program()
    return _cached["nc"]
